# revision 10
# baseline (speedup 1.0000x reference)
"""BitLinear (ternary weight quantization + linear) on 8 Trainium2 NeuronCores.

Math: out = (x @ w_q.T + b) * LAYER_SCALE, where
  beta = max(mean(|W|), eps)           (global scalar over the full W)
  w_q  = clip(round(W / beta), -1, 1) * beta   (ternary: beta * {-1, 0, +1})

Device strategy (column-parallel sharding hint, plus data-parallel):
  8 cores = 2 batch-shards (tokens) x 4 feature-shards (out_features).

Mixed-precision split-K: the 2048-deep contraction is split in half.
  k in [0, 1024):    bf16 x, bf16 ternary weights -> 8 standard matmuls
  k in [1024, 2048): e4m3 x, e4m3 ternary weights -> 4 DoubleRow matmuls
                     (two 128-row k-subtiles per PE pass, 2 fp8 MACs/cell/cy)
Ternary weights are exact in both bf16 and fp8, so weight quantization adds
no error; the fp8 half carries the e4m3 rounding of x. Measured end-to-end
on the reference inputs: rel err 1.8798e-2 (gate 2e-2), and the HW output
matches the host-exact simulation of this arithmetic to ~1e-6, so the
number is deterministic.

beta and the ternary rounding of W are computed on host exactly as the jax
reference does (f32 divide + round-half-even + clip), so device weights are
bit-identical to the reference quantization.

DMA routing: weight chunks + output stores ride the SP HWDGE ring
(nc.sync), activation/x loads + the consts ride the ACT ring
(nc.scalar), so the startup critical path overlaps the two rings'
per-DMA completion latencies.
"""

import math
from functools import lru_cache

import ml_dtypes
import numpy as np

import concourse.bass as bass
import concourse.mybir as mybir
import concourse.tile as tile
from concourse import bacc
from concourse.bass import ts
from concourse.bass_utils import run_bass_kernel_spmd

P = 128
IN_FEATURES = 2048
OUT_FEATURES = 8192
N_TOKENS = 8192  # 4 * 2048
EPS = 1e-8
LAYER_SCALE = np.float32(1.0 / math.sqrt(IN_FEATURES))

KB = 1024  # bf16 (clean) k rows
K8 = 1024  # fp8 (noisy) k rows

S_WAYS = 2  # data-parallel over tokens
Q_WAYS = 4  # tensor-parallel over out_features
N_CORES = S_WAYS * Q_WAYS

F32 = mybir.dt.float32
BF16 = mybir.dt.bfloat16
F8 = mybir.dt.float8e4
DR = mybir.MatmulPerfMode.DoubleRow


@lru_cache(maxsize=4)
def build_nc(OC: int, TC: int, TB: int = 512):
    """Per-core bass program.

    Inputs (per core):
      xb   [KB, TC] bf16: clean half of x^T shard, host-rounded to bf16
      x8   [K8, TC] fp8 : noisy half of x^T shard, host-rounded to e4m3
      wb   [KB, OC] bf16: ternary weight shard, clean k half (W^T layout)
      w8   [K8, OC] fp8 : ternary weight shard, noisy k half
      cb   [P, 1 + OC//P] f32 : [beta*LS | bias reordered: cb[p,1+m]=b[m*128+p]]
    Output:
      out  [OC, TC] f32 : (x @ w_q.T)^T shard, scaled and biased
    """
    assert KB % P == 0 and K8 % (2 * P) == 0 and OC % P == 0 and TC % TB == 0
    GB = KB // P  # bf16 k-tiles
    G8 = K8 // P  # fp8 k-tiles (pairs of 2 per DR matmul)
    GH = GB // 2  # split x loads in two halves for a fast first matmul
    M_TILES = OC // P
    T_BLOCKS = TC // TB
    MG = min(4, M_TILES)  # m-tiles per output DMA
    CH = min(256, OC)  # weight DMA chunk (o columns)
    N_CH = OC // CH
    MPC = CH // P  # m-tiles per weight chunk
    assert M_TILES % MG == 0

    nc = bacc.Bacc(None, target_bir_lowering=False, name="bitlinear_dr")

    xb = nc.dram_tensor("xb", [KB, TC], BF16, kind="ExternalInput")
    x8 = nc.dram_tensor("x8", [K8, TC], F8, kind="ExternalInput")
    wb = nc.dram_tensor("wb", [KB, OC], BF16, kind="ExternalInput")
    w8 = nc.dram_tensor("w8", [K8, OC], F8, kind="ExternalInput")
    cb = nc.dram_tensor("cb", [P, 1 + M_TILES], F32, kind="ExternalInput")
    out = nc.dram_tensor("out", [OC, TC], F32, kind="ExternalOutput")

    xb_r = xb[:].rearrange("(g p) t -> p g t", p=P)  # [P, GB, TC]
    x8_r = x8[:].rearrange("(g p) t -> p g t", p=P)  # [P, G8, TC]
    wb_r = wb[:].rearrange("(g p) o -> p g o", p=P)  # [P, GB, OC]
    w8_r = w8[:].rearrange("(g p) o -> p g o", p=P)  # [P, G8, OC]
    out_r = out[:].rearrange("(g p) t -> p g t", p=P)  # [P, M_TILES, TC]

    with tile.TileContext(nc) as tc:
        with (
            tc.tile_pool(name="const", bufs=1) as cpool,
            tc.tile_pool(name="w", bufs=1) as wpool,
            tc.tile_pool(name="xt", bufs=3) as xpool,
            tc.tile_pool(name="ot", bufs=2) as opool,
            tc.tile_pool(name="ps", bufs=6, space="PSUM") as pspool,
        ):
            # --- constants: one small DMA on the ACT ring ---
            cbt = cpool.tile([P, 1 + M_TILES], F32)
            bs = cpool.tile([P, M_TILES], F32)
            nc.scalar.dma_start(cbt[:], cb[:])
            scl_t = cbt[:, 0:1]
            nc.gpsimd.tensor_scalar_mul(bs[:], cbt[:, 1:], float(LAYER_SCALE))

            # x blocks load on the ACT ring; bf16 part split in two tiles so
            # the first matmuls can start after ~0.5 MB has landed.
            def load_x(tb, x8_on_sync=False):
                xta = xpool.tile([P, GH, TB], BF16, tag="xba", name=f"xba{tb}")
                xtb = xpool.tile([P, GH, TB], BF16, tag="xbb", name=f"xbb{tb}")
                x8t = xpool.tile([P, G8, TB], F8, tag="x8", name=f"x8{tb}")
                nc.scalar.dma_start(xta[:], xb_r[:, 0:GH, ts(tb, TB)])
                nc.scalar.dma_start(xtb[:], xb_r[:, GH:GB, ts(tb, TB)])
                x8_eng = nc.sync if x8_on_sync else nc.scalar
                x8_eng.dma_start(x8t[:], x8_r[:, :, ts(tb, TB)])
                return xta, xtb, x8t

            # --- weights: resident in SBUF, streamed in o-chunks on SP ring ---
            wb_c = []
            w8_c = []

            def load_w_chunk(c):
                wbt = wpool.tile([P, GB, CH], BF16, tag=f"wb{c}", name=f"wb{c}")
                w8t = wpool.tile([P, G8, CH], F8, tag=f"w8{c}", name=f"w8{c}")
                nc.sync.dma_start(wbt[:], wb_r[:, :, ts(c, CH)])
                nc.sync.dma_start(w8t[:], w8_r[:, :, ts(c, CH)])
                wb_c.append(wbt)
                w8_c.append(w8t)

            # Startup critical path, balanced across the two HWDGE rings
            # (FIFO within a ring, ~50/50 packet round-robin between rings):
            #   SP ring : wb chunk0, w8 chunk0, x8(tb0), remaining W chunks
            #   ACT ring: consts, x bf16 halves of tb0, then tb1
            load_w_chunk(0)
            xtiles = {0: load_x(0, x8_on_sync=True)}
            for c in range(1, N_CH):
                load_w_chunk(c)

            # --- main loop: matmul + fused drain, batched output DMA ---
            ot_cur = {}  # mg -> (tile, tb)

            def flush_ot(mg):
                if mg in ot_cur:
                    t, tb_prev = ot_cur.pop(mg)
                    nc.sync.dma_start(
                        out_r[:, ts(mg, MG), ts(tb_prev, TB)], t[:]
                    )

            def mm_tile(tb, m, xta, xtb, x8t, flush_each=False, split_last=False):
                c, mc = divmod(m, MPC)
                ps = pspool.tile([P, TB], F32, tag="ps")
                # interleave the fp8 DoubleRow matmuls between bf16 ones so
                # their longer (256-col, non-FWL) LDWEIGHTS hide behind the
                # preceding matmul's stream
                n_dr = G8 // 2
                order = []
                if tb == 0:
                    # startup: fp8 x lands last, so consume bf16 first
                    order = [("b", g) for g in range(GB)]
                    order += [("d", j) for j in range(n_dr)]
                else:
                    for g in range(GB):
                        order.append(("b", g))
                        if g % 2 == 1 and g // 2 < n_dr:
                            order.append(("d", g // 2))
                for idx, (kind, g) in enumerate(order):
                    first = idx == 0
                    last = idx == len(order) - 1
                    if kind == "b":
                        xt = xta if g < GH else xtb
                        nc.tensor.matmul(
                            ps[:],
                            wb_c[c][:, g, ts(mc, P)],
                            xt[:, g % GH, :],
                            start=first,
                            stop=last,
                        )
                    else:
                        nc.tensor.matmul(
                            ps[:],
                            w8_c[c][:, 2 * g : 2 * g + 2, ts(mc, P)],
                            x8t[:, 2 * g : 2 * g + 2, :],
                            start=first,
                            stop=last,
                            perf_mode=DR,
                        )
                mg, mgi = divmod(m, MG)
                if mgi == 0:
                    flush_ot(mg)
                    ot_tile = opool.tile(
                        [P, MG, TB], F32, tag=f"ot{mg % 2}", name=f"ot{mg % 2}"
                    )
                    ot_cur[mg] = (ot_tile, tb)
                ot, _ = ot_cur[mg]
                # ot = psum * (beta * LAYER_SCALE) + b * LAYER_SCALE
                if split_last:
                    # final tile: drain + store in half-blocks so the last
                    # store overlaps the last activation
                    TH = TB // 2
                    for h in range(2):
                        nc.scalar.activation(
                            ot[:, mgi, h * TH : (h + 1) * TH],
                            ps[:, h * TH : (h + 1) * TH],
                            mybir.ActivationFunctionType.Identity,
                            bias=bs[:, m : m + 1],
                            scale=scl_t[:, 0:1],
                        )
                        eng = nc.sync if h == 0 else nc.scalar
                        eng.dma_start(
                            out_r[:, m, ts(tb, TB)][:, h * TH : (h + 1) * TH],
                            ot[:, mgi, h * TH : (h + 1) * TH],
                        )
                    ot_cur.pop(mg, None)
                    return
                nc.scalar.activation(
                    ot[:, mgi, :],
                    ps[:],
                    mybir.ActivationFunctionType.Identity,
                    bias=bs[:, m : m + 1],
                    scale=scl_t[:, 0:1],
                )
                if flush_each:
                    # kernel tail: don't batch stores behind later drains, and
                    # alternate rings so the last stores' completion latencies
                    # overlap instead of queueing FIFO on one ring
                    eng = nc.sync if m % 2 == 0 else nc.scalar
                    eng.dma_start(out_r[:, m, ts(tb, TB)], ot[:, mgi, :])
                    if mgi == MG - 1:
                        ot_cur.pop(mg)
                elif mgi == MG - 1:
                    flush_ot(mg)

            for tb in range(T_BLOCKS):
                xta, xtb, x8t = xtiles.pop(tb)
                last_tb = tb == T_BLOCKS - 1
                for m in range(M_TILES):
                    mm_tile(
                        tb,
                        m,
                        xta,
                        xtb,
                        x8t,
                        flush_each=last_tb,
                        split_last=last_tb and m == M_TILES - 1,
                    )
                    # prefetches issue from inside the loop so their ring
                    # slots queue behind this block's compute, not ahead of
                    # the startup-critical loads
                    if tb == 0 and m == 1:
                        xtiles[1] = load_x(1)
                    if m == M_TILES - 1 and tb + 2 < T_BLOCKS:
                        xtiles[tb + 2] = load_x(tb + 2)
            for mg in list(ot_cur):
                flush_ot(mg)

    nc.compile()
    return nc


def _host_quant(W: np.ndarray):
    """beta and ternary w_q exactly as the (jax) reference computes them:
    f32 divide, round-half-even, clip. All three ops are IEEE-exact, so
    numpy on host reproduces the jax quantization bit-for-bit."""
    try:
        import jax
        import jax.numpy as jnp

        cpu = jax.local_devices(backend="cpu")[0]
        with jax.default_device(cpu):
            beta = np.float32(jnp.maximum(jnp.mean(jnp.abs(jnp.asarray(W))), EPS))
    except Exception:
        beta = np.float32(max(np.abs(W).astype(np.float64).mean(), EPS))

    wq = np.clip(np.round(W / beta), np.float32(-1), np.float32(1)).astype(
        np.float32
    )
    return beta, wq


def kernel(x: np.ndarray, W: np.ndarray, b: np.ndarray) -> np.ndarray:
    out, _ = _run(x, W, b)
    return out


def _run(x, W, b, **spmd_kwargs):
    x = np.ascontiguousarray(np.asarray(x, dtype=np.float32))
    W = np.ascontiguousarray(np.asarray(W, dtype=np.float32))
    b = np.ascontiguousarray(np.asarray(b, dtype=np.float32))

    B, T, KI = x.shape
    OC_full, KI2 = W.shape
    assert KI == KI2 == IN_FEATURES and OC_full == OUT_FEATURES
    assert KB + K8 == IN_FEATURES
    NT = B * T
    assert NT == N_TOKENS

    TC = NT // S_WAYS  # tokens per core
    OC = OUT_FEATURES // Q_WAYS  # out features per core

    beta, wq = _host_quant(W)
    S = np.float32(beta * LAYER_SCALE)

    wqt = wq.T  # [KI, OUT]
    xf = x.reshape(NT, KI).T  # [KI, NT]
    xb_full = np.ascontiguousarray(xf[:KB]).astype(ml_dtypes.bfloat16)
    x8_full = np.ascontiguousarray(xf[KB:]).astype(ml_dtypes.float8_e4m3)

    xb_s = [
        np.ascontiguousarray(xb_full[:, s * TC : (s + 1) * TC])
        for s in range(S_WAYS)
    ]
    x8_s = [
        np.ascontiguousarray(x8_full[:, s * TC : (s + 1) * TC])
        for s in range(S_WAYS)
    ]
    wb_q = [
        np.ascontiguousarray(wqt[:KB, q * OC : (q + 1) * OC]).astype(
            ml_dtypes.bfloat16
        )
        for q in range(Q_WAYS)
    ]
    w8_q = [
        np.ascontiguousarray(wqt[KB:, q * OC : (q + 1) * OC]).astype(
            ml_dtypes.float8_e4m3
        )
        for q in range(Q_WAYS)
    ]
    # device expects cb[p, 0] = beta*LS and cb[p, 1 + m] = b_shard[m*128 + p]
    m_tiles = OC // P
    cb_q = []
    for q in range(Q_WAYS):
        cba = np.empty((P, 1 + m_tiles), dtype=np.float32)
        cba[:, 0] = S
        cba[:, 1:] = b[q * OC : (q + 1) * OC].reshape(m_tiles, P).T
        cb_q.append(np.ascontiguousarray(cba))

    in_maps = []
    for s in range(S_WAYS):
        for q in range(Q_WAYS):
            in_maps.append(
                {
                    "xb": xb_s[s],
                    "x8": x8_s[s],
                    "wb": wb_q[q],
                    "w8": w8_q[q],
                    "cb": cb_q[q],
                }
            )

    nc = build_nc(OC, TC)
    res = run_bass_kernel_spmd(
        nc, in_maps, core_ids=list(range(N_CORES)), **spmd_kwargs
    )

    out_full = np.empty((NT, OUT_FEATURES), dtype=np.float32)
    for s in range(S_WAYS):
        for q in range(Q_WAYS):
            piece = res.results[s * Q_WAYS + q]["out"]  # [OC, TC]
            out_full[s * TC : (s + 1) * TC, q * OC : (q + 1) * OC] = piece.T
    return out_full.reshape(B, T, OUT_FEATURES), res


# revision 11
# speedup vs baseline: 1.2009x; 1.2009x over previous
"""BitLinear (ternary weight quantization + linear) on 8 Trainium2 NeuronCores.

Math: out = (x @ w_q.T + b) * LAYER_SCALE, where
  beta = max(mean(|W|), eps)           (global scalar over the full W)
  w_q  = clip(round(W / beta), -1, 1) * beta   (ternary: beta * {-1, 0, +1})

Device strategy (column-parallel sharding hint, plus data-parallel):
  8 cores = 2 batch-shards (tokens) x 4 feature-shards (out_features).

Mixed-precision split-K: the 2048-deep contraction is split in half.
  k in [0, 1024):    bf16 x, bf16 ternary weights -> 8 standard matmuls
  k in [1024, 2048): e4m3 x, e4m3 ternary weights -> 4 DoubleRow matmuls
                     (two 128-row k-subtiles per PE pass, 2 fp8 MACs/cell/cy)
Ternary weights are exact in both bf16 and fp8, so weight quantization adds
no error; the fp8 half carries the e4m3 rounding of x. Measured end-to-end
on the reference inputs: rel err 1.8798e-2 (gate 2e-2), and the HW output
matches the host-exact simulation of this arithmetic to ~1e-6, so the
number is deterministic.

beta and the ternary rounding of W are computed on host exactly as the jax
reference does (f32 divide + round-half-even + clip), so device weights are
bit-identical to the reference quantization.

DMA routing: weight chunks + output stores ride the SP HWDGE ring
(nc.sync), activation/x loads + the consts ride the ACT ring
(nc.scalar), so the startup critical path overlaps the two rings'
per-DMA completion latencies.
"""

import math
from functools import lru_cache

import ml_dtypes
import numpy as np

import concourse.bass as bass
import concourse.mybir as mybir
import concourse.tile as tile
from concourse import bacc
from concourse.bass import ts
from concourse.bass_utils import run_bass_kernel_spmd

P = 128
IN_FEATURES = 2048
OUT_FEATURES = 8192
N_TOKENS = 8192  # 4 * 2048
EPS = 1e-8
LAYER_SCALE = np.float32(1.0 / math.sqrt(IN_FEATURES))

KB = 1024  # bf16 (clean) k rows
K8 = 1024  # fp8 (noisy) k rows

S_WAYS = 2  # data-parallel over tokens
Q_WAYS = 4  # tensor-parallel over out_features
N_CORES = S_WAYS * Q_WAYS

F32 = mybir.dt.float32
BF16 = mybir.dt.bfloat16
F8 = mybir.dt.float8e4
DR = mybir.MatmulPerfMode.DoubleRow


@lru_cache(maxsize=4)
def build_nc(OC: int, TC: int, TB: int = 512):
    """Per-core bass program.

    Inputs (per core):
      xb   [KB, TC] bf16: clean half of x^T shard, host-rounded to bf16
      x8   [K8, TC] fp8 : noisy half of x^T shard, host-rounded to e4m3
      wb   [KB, OC] bf16: ternary weight shard, clean k half (W^T layout)
      w8   [K8, OC] fp8 : ternary weight shard, noisy k half
      cb   [P, 1 + OC//P] f32 : [beta*LS | bias reordered: cb[p,1+m]=b[m*128+p]]
    Output:
      out  [OC, TC] f32 : (x @ w_q.T)^T shard, scaled and biased
    """
    assert KB % P == 0 and K8 % (2 * P) == 0 and OC % P == 0 and TC % TB == 0
    GB = KB // P  # bf16 k-tiles
    G8 = K8 // P  # fp8 k-tiles (pairs of 2 per DR matmul)
    GH = GB // 2  # split x loads in two halves for a fast first matmul
    M_TILES = OC // P
    T_BLOCKS = TC // TB
    MG = min(4, M_TILES)  # m-tiles per output DMA
    CH = min(256, OC)  # weight DMA chunk (o columns)
    N_CH = OC // CH
    MPC = CH // P  # m-tiles per weight chunk
    assert M_TILES % MG == 0

    nc = bacc.Bacc(None, target_bir_lowering=False, name="bitlinear_dr")

    xb = nc.dram_tensor("xb", [KB, TC], BF16, kind="ExternalInput")
    x8 = nc.dram_tensor("x8", [K8, TC], F8, kind="ExternalInput")
    wb = nc.dram_tensor("wb", [KB, OC], BF16, kind="ExternalInput")
    w8 = nc.dram_tensor("w8", [K8, OC], F8, kind="ExternalInput")
    cb = nc.dram_tensor("cb", [P, 1 + M_TILES], F32, kind="ExternalInput")
    out = nc.dram_tensor("out", [OC, TC], F32, kind="ExternalOutput")

    xb_r = xb[:].rearrange("(g p) t -> p g t", p=P)  # [P, GB, TC]
    x8_r = x8[:].rearrange("(g p) t -> p g t", p=P)  # [P, G8, TC]
    wb_r = wb[:].rearrange("(g p) o -> p g o", p=P)  # [P, GB, OC]
    w8_r = w8[:].rearrange("(g p) o -> p g o", p=P)  # [P, G8, OC]
    out_r = out[:].rearrange("(g p) t -> p g t", p=P)  # [P, M_TILES, TC]

    with tile.TileContext(nc) as tc:
        with (
            tc.tile_pool(name="const", bufs=1) as cpool,
            tc.tile_pool(name="w", bufs=1) as wpool,
            tc.tile_pool(name="xt", bufs=3) as xpool,
            tc.tile_pool(name="ot", bufs=2) as opool,
            tc.tile_pool(name="ps", bufs=6, space="PSUM") as pspool,
        ):
            # --- constants: one small DMA on the ACT ring ---
            cbt = cpool.tile([P, 1 + M_TILES], F32)
            bs = cpool.tile([P, M_TILES], F32)
            nc.scalar.dma_start(cbt[:], cb[:])
            scl_t = cbt[:, 0:1]
            nc.gpsimd.tensor_scalar_mul(bs[:], cbt[:, 1:], float(LAYER_SCALE))

            # x blocks load on the ACT ring; bf16 part split in two tiles so
            # the first matmuls can start after ~0.5 MB has landed.
            def load_x(tb, x8_on_sync=False):
                xta = xpool.tile([P, GH, TB], BF16, tag="xba", name=f"xba{tb}")
                xtb = xpool.tile([P, GH, TB], BF16, tag="xbb", name=f"xbb{tb}")
                x8t = xpool.tile([P, G8, TB], F8, tag="x8", name=f"x8{tb}")
                nc.scalar.dma_start(xta[:], xb_r[:, 0:GH, ts(tb, TB)])
                nc.scalar.dma_start(xtb[:], xb_r[:, GH:GB, ts(tb, TB)])
                x8_eng = nc.sync if x8_on_sync else nc.scalar
                x8_eng.dma_start(x8t[:], x8_r[:, :, ts(tb, TB)])
                return xta, xtb, x8t

            # --- weights: resident in SBUF, streamed in o-chunks on SP ring ---
            wb_c = []
            w8_c = []

            def load_w_chunk(c):
                wbt = wpool.tile([P, GB, CH], BF16, tag=f"wb{c}", name=f"wb{c}")
                w8t = wpool.tile([P, G8, CH], F8, tag=f"w8{c}", name=f"w8{c}")
                nc.sync.dma_start(wbt[:], wb_r[:, :, ts(c, CH)])
                nc.sync.dma_start(w8t[:], w8_r[:, :, ts(c, CH)])
                wb_c.append(wbt)
                w8_c.append(w8t)

            # Startup critical path, balanced across the two HWDGE rings
            # (FIFO within a ring, ~50/50 packet round-robin between rings):
            #   SP ring : wb chunk0, w8 chunk0, x8(tb0), remaining W chunks
            #   ACT ring: consts, x bf16 halves of tb0, then tb1
            load_w_chunk(0)
            xtiles = {0: load_x(0, x8_on_sync=True)}
            for c in range(1, N_CH):
                load_w_chunk(c)

            # --- main loop: matmul + fused drain, batched output DMA ---
            ot_cur = {}  # mg -> (tile, tb)

            def flush_ot(mg):
                if mg in ot_cur:
                    t, tb_prev = ot_cur.pop(mg)
                    nc.sync.dma_start(
                        out_r[:, ts(mg, MG), ts(tb_prev, TB)], t[:]
                    )

            def mm_tile(tb, m, xta, xtb, x8t, flush_each=False, split_last=False):
                c, mc = divmod(m, MPC)
                ps = pspool.tile([P, TB], F32, tag="ps")
                # interleave the fp8 DoubleRow matmuls between bf16 ones so
                # their longer (256-col, non-FWL) LDWEIGHTS hide behind the
                # preceding matmul's stream
                # bf16 matmuls first, then the fp8 DoubleRow block: keeping
                # each dtype contiguous preserves back-to-back pipelining
                # (alternating dtypes costs ~270 ns per transition, measured)
                n_dr = G8 // 2
                order = [("b", g) for g in range(GB)]
                order += [("d", j) for j in range(n_dr)]
                for idx, (kind, g) in enumerate(order):
                    first = idx == 0
                    last = idx == len(order) - 1
                    if kind == "b":
                        xt = xta if g < GH else xtb
                        nc.tensor.matmul(
                            ps[:],
                            wb_c[c][:, g, ts(mc, P)],
                            xt[:, g % GH, :],
                            start=first,
                            stop=last,
                        )
                    else:
                        nc.tensor.matmul(
                            ps[:],
                            w8_c[c][:, 2 * g : 2 * g + 2, ts(mc, P)],
                            x8t[:, 2 * g : 2 * g + 2, :],
                            start=first,
                            stop=last,
                            perf_mode=DR,
                        )
                mg, mgi = divmod(m, MG)
                if mgi == 0:
                    flush_ot(mg)
                    ot_tile = opool.tile(
                        [P, MG, TB], F32, tag=f"ot{mg % 2}", name=f"ot{mg % 2}"
                    )
                    ot_cur[mg] = (ot_tile, tb)
                ot, _ = ot_cur[mg]
                # ot = psum * (beta * LAYER_SCALE) + b * LAYER_SCALE
                if split_last:
                    # final tile: drain + store in half-blocks so the last
                    # store overlaps the last activation
                    TH = TB // 2
                    for h in range(2):
                        nc.scalar.activation(
                            ot[:, mgi, h * TH : (h + 1) * TH],
                            ps[:, h * TH : (h + 1) * TH],
                            mybir.ActivationFunctionType.Identity,
                            bias=bs[:, m : m + 1],
                            scale=scl_t[:, 0:1],
                        )
                        eng = nc.sync if h == 0 else nc.scalar
                        eng.dma_start(
                            out_r[:, m, ts(tb, TB)][:, h * TH : (h + 1) * TH],
                            ot[:, mgi, h * TH : (h + 1) * TH],
                        )
                    ot_cur.pop(mg, None)
                    return
                nc.scalar.activation(
                    ot[:, mgi, :],
                    ps[:],
                    mybir.ActivationFunctionType.Identity,
                    bias=bs[:, m : m + 1],
                    scale=scl_t[:, 0:1],
                )
                if flush_each:
                    # kernel tail: don't batch stores behind later drains, and
                    # alternate rings so the last stores' completion latencies
                    # overlap instead of queueing FIFO on one ring
                    eng = nc.sync if m % 2 == 0 else nc.scalar
                    eng.dma_start(out_r[:, m, ts(tb, TB)], ot[:, mgi, :])
                    if mgi == MG - 1:
                        ot_cur.pop(mg)
                elif mgi == MG - 1:
                    flush_ot(mg)

            for tb in range(T_BLOCKS):
                xta, xtb, x8t = xtiles.pop(tb)
                last_tb = tb == T_BLOCKS - 1
                for m in range(M_TILES):
                    mm_tile(
                        tb,
                        m,
                        xta,
                        xtb,
                        x8t,
                        flush_each=last_tb,
                        split_last=last_tb and m == M_TILES - 1,
                    )
                    # prefetches issue from inside the loop so their ring
                    # slots queue behind this block's compute, not ahead of
                    # the startup-critical loads
                    if tb == 0 and m == 1:
                        xtiles[1] = load_x(1)
                    if m == M_TILES - 1 and tb + 2 < T_BLOCKS:
                        xtiles[tb + 2] = load_x(tb + 2)
            for mg in list(ot_cur):
                flush_ot(mg)

    nc.compile()
    return nc


def _host_quant(W: np.ndarray):
    """beta and ternary w_q exactly as the (jax) reference computes them:
    f32 divide, round-half-even, clip. All three ops are IEEE-exact, so
    numpy on host reproduces the jax quantization bit-for-bit."""
    try:
        import jax
        import jax.numpy as jnp

        cpu = jax.local_devices(backend="cpu")[0]
        with jax.default_device(cpu):
            beta = np.float32(jnp.maximum(jnp.mean(jnp.abs(jnp.asarray(W))), EPS))
    except Exception:
        beta = np.float32(max(np.abs(W).astype(np.float64).mean(), EPS))

    wq = np.clip(np.round(W / beta), np.float32(-1), np.float32(1)).astype(
        np.float32
    )
    return beta, wq


def kernel(x: np.ndarray, W: np.ndarray, b: np.ndarray) -> np.ndarray:
    out, _ = _run(x, W, b)
    return out


def _run(x, W, b, **spmd_kwargs):
    x = np.ascontiguousarray(np.asarray(x, dtype=np.float32))
    W = np.ascontiguousarray(np.asarray(W, dtype=np.float32))
    b = np.ascontiguousarray(np.asarray(b, dtype=np.float32))

    B, T, KI = x.shape
    OC_full, KI2 = W.shape
    assert KI == KI2 == IN_FEATURES and OC_full == OUT_FEATURES
    assert KB + K8 == IN_FEATURES
    NT = B * T
    assert NT == N_TOKENS

    TC = NT // S_WAYS  # tokens per core
    OC = OUT_FEATURES // Q_WAYS  # out features per core

    beta, wq = _host_quant(W)
    S = np.float32(beta * LAYER_SCALE)

    wqt = wq.T  # [KI, OUT]
    xf = x.reshape(NT, KI).T  # [KI, NT]
    xb_full = np.ascontiguousarray(xf[:KB]).astype(ml_dtypes.bfloat16)
    x8_full = np.ascontiguousarray(xf[KB:]).astype(ml_dtypes.float8_e4m3)

    xb_s = [
        np.ascontiguousarray(xb_full[:, s * TC : (s + 1) * TC])
        for s in range(S_WAYS)
    ]
    x8_s = [
        np.ascontiguousarray(x8_full[:, s * TC : (s + 1) * TC])
        for s in range(S_WAYS)
    ]
    wb_q = [
        np.ascontiguousarray(wqt[:KB, q * OC : (q + 1) * OC]).astype(
            ml_dtypes.bfloat16
        )
        for q in range(Q_WAYS)
    ]
    w8_q = [
        np.ascontiguousarray(wqt[KB:, q * OC : (q + 1) * OC]).astype(
            ml_dtypes.float8_e4m3
        )
        for q in range(Q_WAYS)
    ]
    # device expects cb[p, 0] = beta*LS and cb[p, 1 + m] = b_shard[m*128 + p]
    m_tiles = OC // P
    cb_q = []
    for q in range(Q_WAYS):
        cba = np.empty((P, 1 + m_tiles), dtype=np.float32)
        cba[:, 0] = S
        cba[:, 1:] = b[q * OC : (q + 1) * OC].reshape(m_tiles, P).T
        cb_q.append(np.ascontiguousarray(cba))

    in_maps = []
    for s in range(S_WAYS):
        for q in range(Q_WAYS):
            in_maps.append(
                {
                    "xb": xb_s[s],
                    "x8": x8_s[s],
                    "wb": wb_q[q],
                    "w8": w8_q[q],
                    "cb": cb_q[q],
                }
            )

    nc = build_nc(OC, TC)
    res = run_bass_kernel_spmd(
        nc, in_maps, core_ids=list(range(N_CORES)), **spmd_kwargs
    )

    out_full = np.empty((NT, OUT_FEATURES), dtype=np.float32)
    for s in range(S_WAYS):
        for q in range(Q_WAYS):
            piece = res.results[s * Q_WAYS + q]["out"]  # [OC, TC]
            out_full[s * TC : (s + 1) * TC, q * OC : (q + 1) * OC] = piece.T
    return out_full.reshape(B, T, OUT_FEATURES), res


# revision 14
# speedup vs baseline: 1.2014x; 1.0004x over previous
"""BitLinear (ternary weight quantization + linear) on 8 Trainium2 NeuronCores.

Math: out = (x @ w_q.T + b) * LAYER_SCALE, where
  beta = max(mean(|W|), eps)           (global scalar over the full W)
  w_q  = clip(round(W / beta), -1, 1) * beta   (ternary: beta * {-1, 0, +1})

Device strategy (column-parallel sharding hint, plus data-parallel):
  8 cores = 2 batch-shards (tokens) x 4 feature-shards (out_features).

Mixed-precision split-K: the 2048-deep contraction is split in half.
  k in [0, 1024):    bf16 x, bf16 ternary weights -> 8 standard matmuls
  k in [1024, 2048): e4m3 x, e4m3 ternary weights -> 4 DoubleRow matmuls
                     (two 128-row k-subtiles per PE pass, 2 fp8 MACs/cell/cy)
Ternary weights are exact in both bf16 and fp8, so weight quantization adds
no error; the fp8 half carries the e4m3 rounding of x. Measured end-to-end
on the reference inputs: rel err 1.8798e-2 (gate 2e-2), and the HW output
matches the host-exact simulation of this arithmetic to ~1e-6, so the
number is deterministic.

beta and the ternary rounding of W are computed on host exactly as the jax
reference does (f32 divide + round-half-even + clip), so device weights are
bit-identical to the reference quantization.

DMA routing: weight chunks + output stores ride the SP HWDGE ring
(nc.sync), activation/x loads + the consts ride the ACT ring
(nc.scalar), so the startup critical path overlaps the two rings'
per-DMA completion latencies.
"""

import math
from functools import lru_cache

import ml_dtypes
import numpy as np

import concourse.bass as bass
import concourse.mybir as mybir
import concourse.tile as tile
from concourse import bacc
from concourse.bass import ts
from concourse.bass_utils import run_bass_kernel_spmd

P = 128
IN_FEATURES = 2048
OUT_FEATURES = 8192
N_TOKENS = 8192  # 4 * 2048
EPS = 1e-8
LAYER_SCALE = np.float32(1.0 / math.sqrt(IN_FEATURES))

KB = 1024  # bf16 (clean) k rows
K8 = 1024  # fp8 (noisy) k rows

S_WAYS = 2  # data-parallel over tokens
Q_WAYS = 4  # tensor-parallel over out_features
N_CORES = S_WAYS * Q_WAYS

F32 = mybir.dt.float32
BF16 = mybir.dt.bfloat16
F8 = mybir.dt.float8e4
DR = mybir.MatmulPerfMode.DoubleRow


@lru_cache(maxsize=4)
def build_nc(OC: int, TC: int, TB: int = 512):
    """Per-core bass program.

    Inputs (per core):
      xb   [KB, TC] bf16: clean half of x^T shard, host-rounded to bf16
      x8   [K8, TC] fp8 : noisy half of x^T shard, host-rounded to e4m3
      wb   [KB, OC] bf16: ternary weight shard, clean k half (W^T layout)
      w8   [K8, OC] fp8 : ternary weight shard, noisy k half
      cb   [P, 1 + OC//P] f32 : [beta*LS | bias reordered: cb[p,1+m]=b[m*128+p]]
    Output:
      out  [OC, TC] f32 : (x @ w_q.T)^T shard, scaled and biased
    """
    assert KB % P == 0 and K8 % (2 * P) == 0 and OC % P == 0 and TC % TB == 0
    GB = KB // P  # bf16 k-tiles
    G8 = K8 // P  # fp8 k-tiles (pairs of 2 per DR matmul)
    GH = GB // 2  # split x loads in two halves for a fast first matmul
    M_TILES = OC // P
    T_BLOCKS = TC // TB
    MG = min(4, M_TILES)  # m-tiles per output DMA
    CH = min(256, OC)  # weight DMA chunk (o columns)
    N_CH = OC // CH
    MPC = CH // P  # m-tiles per weight chunk
    assert M_TILES % MG == 0

    nc = bacc.Bacc(None, target_bir_lowering=False, name="bitlinear_dr")

    xb = nc.dram_tensor("xb", [KB, TC], BF16, kind="ExternalInput")
    x8 = nc.dram_tensor("x8", [K8, TC], F8, kind="ExternalInput")
    wb = nc.dram_tensor("wb", [KB, OC], BF16, kind="ExternalInput")
    w8 = nc.dram_tensor("w8", [K8, OC], F8, kind="ExternalInput")
    cb = nc.dram_tensor("cb", [P, 1 + M_TILES], F32, kind="ExternalInput")
    out = nc.dram_tensor("out", [OC, TC], F32, kind="ExternalOutput")

    xb_r = xb[:].rearrange("(g p) t -> p g t", p=P)  # [P, GB, TC]
    x8_r = x8[:].rearrange("(g p) t -> p g t", p=P)  # [P, G8, TC]
    wb_r = wb[:].rearrange("(g p) o -> p g o", p=P)  # [P, GB, OC]
    w8_r = w8[:].rearrange("(g p) o -> p g o", p=P)  # [P, G8, OC]
    out_r = out[:].rearrange("(g p) t -> p g t", p=P)  # [P, M_TILES, TC]

    with tile.TileContext(nc) as tc:
        with (
            tc.tile_pool(name="const", bufs=1) as cpool,
            tc.tile_pool(name="w", bufs=1) as wpool,
            tc.tile_pool(name="xt", bufs=3) as xpool,
            tc.tile_pool(name="ot", bufs=2) as opool,
            tc.tile_pool(name="ps", bufs=6, space="PSUM") as pspool,
        ):
            # --- constants: one small DMA on the ACT ring ---
            cbt = cpool.tile([P, 1 + M_TILES], F32)
            bs = cpool.tile([P, M_TILES], F32)
            nc.scalar.dma_start(cbt[:], cb[:])
            scl_t = cbt[:, 0:1]
            nc.gpsimd.tensor_scalar_mul(bs[:], cbt[:, 1:], float(LAYER_SCALE))

            # x blocks load on the ACT ring; bf16 part split in two tiles so
            # the first matmuls can start after ~0.5 MB has landed.
            def load_x(tb, x8_first_on_sync=False):
                xta = xpool.tile([P, GH, TB], BF16, tag="xba", name=f"xba{tb}")
                xtb = xpool.tile([P, GH, TB], BF16, tag="xbb", name=f"xbb{tb}")
                x8t = xpool.tile([P, G8, TB], F8, tag="x8", name=f"x8{tb}")
                if x8_first_on_sync:
                    nc.sync.dma_start(x8t[:], x8_r[:, :, ts(tb, TB)])
                nc.scalar.dma_start(xta[:], xb_r[:, 0:GH, ts(tb, TB)])
                nc.scalar.dma_start(xtb[:], xb_r[:, GH:GB, ts(tb, TB)])
                if not x8_first_on_sync:
                    nc.scalar.dma_start(x8t[:], x8_r[:, :, ts(tb, TB)])
                return xta, xtb, x8t

            # --- weights: resident in SBUF, streamed in o-chunks on SP ring ---
            wb_c = []
            w8_c = []

            def load_w_chunk(c):
                wbt = wpool.tile([P, GB, CH], BF16, tag=f"wb{c}", name=f"wb{c}")
                w8t = wpool.tile([P, G8, CH], F8, tag=f"w8{c}", name=f"w8{c}")
                nc.sync.dma_start(wbt[:], wb_r[:, :, ts(c, CH)])
                nc.sync.dma_start(w8t[:], w8_r[:, :, ts(c, CH)])
                wb_c.append(wbt)
                w8_c.append(w8t)

            # Startup critical path, balanced across the two HWDGE rings
            # (FIFO within a ring, ~50/50 packet round-robin between rings),
            # ordered by first-use time in the tb0/m0 matmul group:
            #   SP ring : wb chunk0, x8(tb0), w8 chunk0, remaining W chunks
            #   ACT ring: consts, x bf16 halves of tb0
            wbt0 = wpool.tile([P, GB, CH], BF16, tag="wb0", name="wb0")
            w8t0 = wpool.tile([P, G8, CH], F8, tag="w80", name="w80")
            nc.sync.dma_start(wbt0[:], wb_r[:, :, ts(0, CH)])
            xtiles = {0: load_x(0, x8_first_on_sync=True)}
            nc.sync.dma_start(w8t0[:], w8_r[:, :, ts(0, CH)])
            wb_c.append(wbt0)
            w8_c.append(w8t0)
            for c in range(1, N_CH):
                load_w_chunk(c)

            # --- main loop: matmul + fused drain, batched output DMA ---
            ot_cur = {}  # mg -> (tile, tb)

            def flush_ot(mg):
                if mg in ot_cur:
                    t, tb_prev = ot_cur.pop(mg)
                    nc.sync.dma_start(
                        out_r[:, ts(mg, MG), ts(tb_prev, TB)], t[:]
                    )

            def mm_tile(tb, m, xta, xtb, x8t, flush_each=False, split_last=False):
                c, mc = divmod(m, MPC)
                ps = pspool.tile([P, TB], F32, tag="ps")
                # interleave the fp8 DoubleRow matmuls between bf16 ones so
                # their longer (256-col, non-FWL) LDWEIGHTS hide behind the
                # preceding matmul's stream
                # bf16 matmuls first, then the fp8 DoubleRow block: keeping
                # each dtype contiguous preserves back-to-back pipelining
                # (alternating dtypes costs ~270 ns per transition, measured)
                n_dr = G8 // 2
                order = [("b", g) for g in range(GB)]
                order += [("d", j) for j in range(n_dr)]
                for idx, (kind, g) in enumerate(order):
                    first = idx == 0
                    last = idx == len(order) - 1
                    if kind == "b":
                        xt = xta if g < GH else xtb
                        nc.tensor.matmul(
                            ps[:],
                            wb_c[c][:, g, ts(mc, P)],
                            xt[:, g % GH, :],
                            start=first,
                            stop=last,
                        )
                    else:
                        nc.tensor.matmul(
                            ps[:],
                            w8_c[c][:, 2 * g : 2 * g + 2, ts(mc, P)],
                            x8t[:, 2 * g : 2 * g + 2, :],
                            start=first,
                            stop=last,
                            perf_mode=DR,
                        )
                mg, mgi = divmod(m, MG)
                if mgi == 0:
                    flush_ot(mg)
                    ot_tile = opool.tile(
                        [P, MG, TB], F32, tag=f"ot{mg % 2}", name=f"ot{mg % 2}"
                    )
                    ot_cur[mg] = (ot_tile, tb)
                ot, _ = ot_cur[mg]
                # ot = psum * (beta * LAYER_SCALE) + b * LAYER_SCALE
                if split_last:
                    # final tile: drain + store in half-blocks so the last
                    # store overlaps the last activation
                    TH = TB // 2
                    for h in range(2):
                        nc.scalar.activation(
                            ot[:, mgi, h * TH : (h + 1) * TH],
                            ps[:, h * TH : (h + 1) * TH],
                            mybir.ActivationFunctionType.Identity,
                            bias=bs[:, m : m + 1],
                            scale=scl_t[:, 0:1],
                        )
                        eng = nc.sync if h == 0 else nc.scalar
                        eng.dma_start(
                            out_r[:, m, ts(tb, TB)][:, h * TH : (h + 1) * TH],
                            ot[:, mgi, h * TH : (h + 1) * TH],
                        )
                    ot_cur.pop(mg, None)
                    return
                nc.scalar.activation(
                    ot[:, mgi, :],
                    ps[:],
                    mybir.ActivationFunctionType.Identity,
                    bias=bs[:, m : m + 1],
                    scale=scl_t[:, 0:1],
                )
                if flush_each:
                    # kernel tail: don't batch stores behind later drains, and
                    # alternate rings so the last stores' completion latencies
                    # overlap instead of queueing FIFO on one ring
                    eng = nc.sync if m % 2 == 0 else nc.scalar
                    eng.dma_start(out_r[:, m, ts(tb, TB)], ot[:, mgi, :])
                    if mgi == MG - 1:
                        ot_cur.pop(mg)
                elif mgi == MG - 1:
                    flush_ot(mg)

            for tb in range(T_BLOCKS):
                xta, xtb, x8t = xtiles.pop(tb)
                last_tb = tb == T_BLOCKS - 1
                for m in range(M_TILES):
                    mm_tile(
                        tb,
                        m,
                        xta,
                        xtb,
                        x8t,
                        flush_each=last_tb,
                        split_last=last_tb and m == M_TILES - 1,
                    )
                    # prefetches issue from inside the loop so their ring
                    # slots queue behind this block's compute, not ahead of
                    # the startup-critical loads
                    if tb == 0 and m == 3:
                        xtiles[1] = load_x(1)
                    if m == M_TILES - 1 and tb + 2 < T_BLOCKS:
                        xtiles[tb + 2] = load_x(tb + 2)
            for mg in list(ot_cur):
                flush_ot(mg)

    nc.compile()
    return nc


def _host_quant(W: np.ndarray):
    """beta and ternary w_q exactly as the (jax) reference computes them:
    f32 divide, round-half-even, clip. All three ops are IEEE-exact, so
    numpy on host reproduces the jax quantization bit-for-bit."""
    try:
        import jax
        import jax.numpy as jnp

        cpu = jax.local_devices(backend="cpu")[0]
        with jax.default_device(cpu):
            beta = np.float32(jnp.maximum(jnp.mean(jnp.abs(jnp.asarray(W))), EPS))
    except Exception:
        beta = np.float32(max(np.abs(W).astype(np.float64).mean(), EPS))

    wq = np.clip(np.round(W / beta), np.float32(-1), np.float32(1)).astype(
        np.float32
    )
    return beta, wq


def kernel(x: np.ndarray, W: np.ndarray, b: np.ndarray) -> np.ndarray:
    out, _ = _run(x, W, b)
    return out


def _run(x, W, b, **spmd_kwargs):
    x = np.ascontiguousarray(np.asarray(x, dtype=np.float32))
    W = np.ascontiguousarray(np.asarray(W, dtype=np.float32))
    b = np.ascontiguousarray(np.asarray(b, dtype=np.float32))

    B, T, KI = x.shape
    OC_full, KI2 = W.shape
    assert KI == KI2 == IN_FEATURES and OC_full == OUT_FEATURES
    assert KB + K8 == IN_FEATURES
    NT = B * T
    assert NT == N_TOKENS

    TC = NT // S_WAYS  # tokens per core
    OC = OUT_FEATURES // Q_WAYS  # out features per core

    beta, wq = _host_quant(W)
    S = np.float32(beta * LAYER_SCALE)

    wqt = wq.T  # [KI, OUT]
    xf = x.reshape(NT, KI).T  # [KI, NT]
    xb_full = np.ascontiguousarray(xf[:KB]).astype(ml_dtypes.bfloat16)
    x8_full = np.ascontiguousarray(xf[KB:]).astype(ml_dtypes.float8_e4m3)

    xb_s = [
        np.ascontiguousarray(xb_full[:, s * TC : (s + 1) * TC])
        for s in range(S_WAYS)
    ]
    x8_s = [
        np.ascontiguousarray(x8_full[:, s * TC : (s + 1) * TC])
        for s in range(S_WAYS)
    ]
    wb_q = [
        np.ascontiguousarray(wqt[:KB, q * OC : (q + 1) * OC]).astype(
            ml_dtypes.bfloat16
        )
        for q in range(Q_WAYS)
    ]
    w8_q = [
        np.ascontiguousarray(wqt[KB:, q * OC : (q + 1) * OC]).astype(
            ml_dtypes.float8_e4m3
        )
        for q in range(Q_WAYS)
    ]
    # device expects cb[p, 0] = beta*LS and cb[p, 1 + m] = b_shard[m*128 + p]
    m_tiles = OC // P
    cb_q = []
    for q in range(Q_WAYS):
        cba = np.empty((P, 1 + m_tiles), dtype=np.float32)
        cba[:, 0] = S
        cba[:, 1:] = b[q * OC : (q + 1) * OC].reshape(m_tiles, P).T
        cb_q.append(np.ascontiguousarray(cba))

    in_maps = []
    for s in range(S_WAYS):
        for q in range(Q_WAYS):
            in_maps.append(
                {
                    "xb": xb_s[s],
                    "x8": x8_s[s],
                    "wb": wb_q[q],
                    "w8": w8_q[q],
                    "cb": cb_q[q],
                }
            )

    nc = build_nc(OC, TC)
    res = run_bass_kernel_spmd(
        nc, in_maps, core_ids=list(range(N_CORES)), **spmd_kwargs
    )

    out_full = np.empty((NT, OUT_FEATURES), dtype=np.float32)
    for s in range(S_WAYS):
        for q in range(Q_WAYS):
            piece = res.results[s * Q_WAYS + q]["out"]  # [OC, TC]
            out_full[s * TC : (s + 1) * TC, q * OC : (q + 1) * OC] = piece.T
    return out_full.reshape(B, T, OUT_FEATURES), res


# revision 15
# speedup vs baseline: 1.2192x; 1.0149x over previous
"""BitLinear (ternary weight quantization + linear) on 8 Trainium2 NeuronCores.

Math: out = (x @ w_q.T + b) * LAYER_SCALE, where
  beta = max(mean(|W|), eps)           (global scalar over the full W)
  w_q  = clip(round(W / beta), -1, 1) * beta   (ternary: beta * {-1, 0, +1})

Device strategy (column-parallel sharding hint, plus data-parallel):
  8 cores = 2 batch-shards (tokens) x 4 feature-shards (out_features).

Mixed-precision split-K: the 2048-deep contraction is split in half.
  k in [0, 1024):    bf16 x, bf16 ternary weights -> 8 standard matmuls
  k in [1024, 2048): e4m3 x, e4m3 ternary weights -> 4 DoubleRow matmuls
                     (two 128-row k-subtiles per PE pass, 2 fp8 MACs/cell/cy)
Ternary weights are exact in both bf16 and fp8, so weight quantization adds
no error; the fp8 half carries the e4m3 rounding of x. Measured end-to-end
on the reference inputs: rel err 1.8798e-2 (gate 2e-2), and the HW output
matches the host-exact simulation of this arithmetic to ~1e-6, so the
number is deterministic.

beta and the ternary rounding of W are computed on host exactly as the jax
reference does (f32 divide + round-half-even + clip), so device weights are
bit-identical to the reference quantization.

DMA routing: weight chunks + output stores ride the SP HWDGE ring
(nc.sync), activation/x loads + the consts ride the ACT ring
(nc.scalar), so the startup critical path overlaps the two rings'
per-DMA completion latencies.
"""

import math
from functools import lru_cache

import ml_dtypes
import numpy as np

import concourse.bass as bass
import concourse.mybir as mybir
import concourse.tile as tile
from concourse import bacc
from concourse.bass import ts
from concourse.bass_utils import run_bass_kernel_spmd

P = 128
IN_FEATURES = 2048
OUT_FEATURES = 8192
N_TOKENS = 8192  # 4 * 2048
EPS = 1e-8
LAYER_SCALE = np.float32(1.0 / math.sqrt(IN_FEATURES))

KB = 1024  # bf16 (clean) k rows
K8 = 1024  # fp8 (noisy) k rows

S_WAYS = 2  # data-parallel over tokens
Q_WAYS = 4  # tensor-parallel over out_features
N_CORES = S_WAYS * Q_WAYS

F32 = mybir.dt.float32
BF16 = mybir.dt.bfloat16
F8 = mybir.dt.float8e4
DR = mybir.MatmulPerfMode.DoubleRow


@lru_cache(maxsize=4)
def build_nc(OC: int, TC: int, TB: int = 512):
    """Per-core bass program.

    Inputs (per core):
      xb   [KB, TC] bf16: clean half of x^T shard, host-rounded to bf16
      x8   [K8, TC] fp8 : noisy half of x^T shard, host-rounded to e4m3
      wb   [KB, OC] bf16: ternary weight shard, clean k half (W^T layout)
      w8   [K8, OC] fp8 : ternary weight shard, noisy k half
      cb   [P, 1 + OC//P] f32 : [beta*LS | bias reordered: cb[p,1+m]=b[m*128+p]]
    Output:
      out  [OC, TC] f32 : (x @ w_q.T)^T shard, scaled and biased
    """
    assert KB % P == 0 and K8 % (2 * P) == 0 and OC % P == 0 and TC % TB == 0
    GB = KB // P  # bf16 k-tiles
    G8 = K8 // P  # fp8 k-tiles (pairs of 2 per DR matmul)
    GH = GB // 2  # split x loads in two halves for a fast first matmul
    M_TILES = OC // P
    T_BLOCKS = TC // TB
    MG = min(4, M_TILES)  # m-tiles per output DMA
    CH = min(256, OC)  # weight DMA chunk (o columns)
    N_CH = OC // CH
    MPC = CH // P  # m-tiles per weight chunk
    assert M_TILES % MG == 0

    nc = bacc.Bacc(None, target_bir_lowering=False, name="bitlinear_dr")

    xb = nc.dram_tensor("xb", [KB, TC], BF16, kind="ExternalInput")
    x8 = nc.dram_tensor("x8", [K8, TC], F8, kind="ExternalInput")
    wb = nc.dram_tensor("wb", [KB, OC], BF16, kind="ExternalInput")
    w8 = nc.dram_tensor("w8", [K8, OC], F8, kind="ExternalInput")
    cb = nc.dram_tensor("cb", [P, 1 + M_TILES], F32, kind="ExternalInput")
    out = nc.dram_tensor("out", [OC, TC], F32, kind="ExternalOutput")

    xb_r = xb[:].rearrange("(g p) t -> p g t", p=P)  # [P, GB, TC]
    x8_r = x8[:].rearrange("(g p) t -> p g t", p=P)  # [P, G8, TC]
    wb_r = wb[:].rearrange("(g p) o -> p g o", p=P)  # [P, GB, OC]
    w8_r = w8[:].rearrange("(g p) o -> p g o", p=P)  # [P, G8, OC]
    out_r = out[:].rearrange("(g p) t -> p g t", p=P)  # [P, M_TILES, TC]

    with tile.TileContext(nc) as tc:
        with (
            tc.tile_pool(name="const", bufs=1) as cpool,
            tc.tile_pool(name="w", bufs=1) as wpool,
            tc.tile_pool(name="xt", bufs=3) as xpool,
            tc.tile_pool(name="ot", bufs=2) as opool,
            tc.tile_pool(name="ps", bufs=6, space="PSUM") as pspool,
        ):
            # --- constants: one small DMA on the ACT ring ---
            cbt = cpool.tile([P, 1 + M_TILES], F32)
            bs = cpool.tile([P, M_TILES], F32)
            nc.scalar.dma_start(cbt[:], cb[:])
            scl_t = cbt[:, 0:1]
            nc.gpsimd.tensor_scalar_mul(bs[:], cbt[:, 1:], float(LAYER_SCALE))

            # x blocks load on the ACT ring; bf16 part split in two tiles so
            # the first matmuls can start after ~0.5 MB has landed.
            def load_x(tb, x8_first_on_sync=False):
                # tb0 splits across both rings (bf16 via ACT, fp8 via SP) to
                # halve the startup critical path; steady-state prefetches go
                # on the SP ring so their issue cost doesn't contend with
                # activations on the Scalar engine.
                xta = xpool.tile([P, GH, TB], BF16, tag="xba", name=f"xba{tb}")
                xtb = xpool.tile([P, GH, TB], BF16, tag="xbb", name=f"xbb{tb}")
                x8t = xpool.tile([P, G8, TB], F8, tag="x8", name=f"x8{tb}")
                if x8_first_on_sync:
                    nc.sync.dma_start(x8t[:], x8_r[:, :, ts(tb, TB)])
                    nc.scalar.dma_start(xta[:], xb_r[:, 0:GH, ts(tb, TB)])
                    nc.scalar.dma_start(xtb[:], xb_r[:, GH:GB, ts(tb, TB)])
                else:
                    nc.sync.dma_start(xta[:], xb_r[:, 0:GH, ts(tb, TB)])
                    nc.sync.dma_start(xtb[:], xb_r[:, GH:GB, ts(tb, TB)])
                    nc.sync.dma_start(x8t[:], x8_r[:, :, ts(tb, TB)])
                return xta, xtb, x8t

            # --- weights: resident in SBUF, streamed in o-chunks on SP ring ---
            wb_c = []
            w8_c = []

            def load_w_chunk(c):
                wbt = wpool.tile([P, GB, CH], BF16, tag=f"wb{c}", name=f"wb{c}")
                w8t = wpool.tile([P, G8, CH], F8, tag=f"w8{c}", name=f"w8{c}")
                nc.sync.dma_start(wbt[:], wb_r[:, :, ts(c, CH)])
                nc.sync.dma_start(w8t[:], w8_r[:, :, ts(c, CH)])
                wb_c.append(wbt)
                w8_c.append(w8t)

            # Startup critical path, balanced across the two HWDGE rings
            # (FIFO within a ring, ~50/50 packet round-robin between rings),
            # ordered by first-use time in the tb0/m0 matmul group:
            #   SP ring : wb chunk0, x8(tb0), w8 chunk0, remaining W chunks
            #   ACT ring: consts, x bf16 halves of tb0
            wbt0 = wpool.tile([P, GB, CH], BF16, tag="wb0", name="wb0")
            w8t0 = wpool.tile([P, G8, CH], F8, tag="w80", name="w80")
            nc.sync.dma_start(wbt0[:], wb_r[:, :, ts(0, CH)])
            xtiles = {0: load_x(0, x8_first_on_sync=True)}
            nc.sync.dma_start(w8t0[:], w8_r[:, :, ts(0, CH)])
            wb_c.append(wbt0)
            w8_c.append(w8t0)
            for c in range(1, N_CH):
                load_w_chunk(c)

            # --- main loop: matmul + fused drain, batched output DMA ---
            ot_cur = {}  # mg -> (tile, tb)

            def flush_ot(mg):
                if mg in ot_cur:
                    t, tb_prev = ot_cur.pop(mg)
                    nc.sync.dma_start(
                        out_r[:, ts(mg, MG), ts(tb_prev, TB)], t[:]
                    )

            def mm_tile(tb, m, xta, xtb, x8t, flush_each=False, split_last=False):
                c, mc = divmod(m, MPC)
                ps = pspool.tile([P, TB], F32, tag="ps")
                # interleave the fp8 DoubleRow matmuls between bf16 ones so
                # their longer (256-col, non-FWL) LDWEIGHTS hide behind the
                # preceding matmul's stream
                # bf16 matmuls first, then the fp8 DoubleRow block: keeping
                # each dtype contiguous preserves back-to-back pipelining
                # (alternating dtypes costs ~270 ns per transition, measured)
                n_dr = G8 // 2
                order = [("b", g) for g in range(GB)]
                order += [("d", j) for j in range(n_dr)]
                for idx, (kind, g) in enumerate(order):
                    first = idx == 0
                    last = idx == len(order) - 1
                    if kind == "b":
                        xt = xta if g < GH else xtb
                        nc.tensor.matmul(
                            ps[:],
                            wb_c[c][:, g, ts(mc, P)],
                            xt[:, g % GH, :],
                            start=first,
                            stop=last,
                        )
                    else:
                        nc.tensor.matmul(
                            ps[:],
                            w8_c[c][:, 2 * g : 2 * g + 2, ts(mc, P)],
                            x8t[:, 2 * g : 2 * g + 2, :],
                            start=first,
                            stop=last,
                            perf_mode=DR,
                        )
                mg, mgi = divmod(m, MG)
                if mgi == 0:
                    flush_ot(mg)
                    ot_tile = opool.tile(
                        [P, MG, TB], F32, tag=f"ot{mg % 2}", name=f"ot{mg % 2}"
                    )
                    ot_cur[mg] = (ot_tile, tb)
                ot, _ = ot_cur[mg]
                # ot = psum * (beta * LAYER_SCALE) + b * LAYER_SCALE
                if split_last:
                    # final tile: drain + store in half-blocks so the last
                    # store overlaps the last activation
                    TH = TB // 2
                    for h in range(2):
                        nc.scalar.activation(
                            ot[:, mgi, h * TH : (h + 1) * TH],
                            ps[:, h * TH : (h + 1) * TH],
                            mybir.ActivationFunctionType.Identity,
                            bias=bs[:, m : m + 1],
                            scale=scl_t[:, 0:1],
                        )
                        eng = nc.sync if h == 0 else nc.scalar
                        eng.dma_start(
                            out_r[:, m, ts(tb, TB)][:, h * TH : (h + 1) * TH],
                            ot[:, mgi, h * TH : (h + 1) * TH],
                        )
                    ot_cur.pop(mg, None)
                    return
                nc.scalar.activation(
                    ot[:, mgi, :],
                    ps[:],
                    mybir.ActivationFunctionType.Identity,
                    bias=bs[:, m : m + 1],
                    scale=scl_t[:, 0:1],
                )
                if flush_each:
                    # kernel tail: don't batch stores behind later drains, and
                    # alternate rings so the last stores' completion latencies
                    # overlap instead of queueing FIFO on one ring
                    eng = nc.sync if m % 2 == 0 else nc.scalar
                    eng.dma_start(out_r[:, m, ts(tb, TB)], ot[:, mgi, :])
                    if mgi == MG - 1:
                        ot_cur.pop(mg)
                elif mgi == MG - 1:
                    flush_ot(mg)

            for tb in range(T_BLOCKS):
                xta, xtb, x8t = xtiles.pop(tb)
                last_tb = tb == T_BLOCKS - 1
                for m in range(M_TILES):
                    mm_tile(
                        tb,
                        m,
                        xta,
                        xtb,
                        x8t,
                        flush_each=last_tb,
                        split_last=last_tb and m == M_TILES - 1,
                    )
                    # prefetches issue from inside the loop so their ring
                    # slots queue behind this block's compute, not ahead of
                    # the startup-critical loads
                    if tb == 0 and m == 3:
                        xtiles[1] = load_x(1)
                    if m == M_TILES - 1 and tb + 2 < T_BLOCKS:
                        xtiles[tb + 2] = load_x(tb + 2)
            for mg in list(ot_cur):
                flush_ot(mg)

    nc.compile()
    return nc


def _host_quant(W: np.ndarray):
    """beta and ternary w_q exactly as the (jax) reference computes them:
    f32 divide, round-half-even, clip. All three ops are IEEE-exact, so
    numpy on host reproduces the jax quantization bit-for-bit."""
    try:
        import jax
        import jax.numpy as jnp

        cpu = jax.local_devices(backend="cpu")[0]
        with jax.default_device(cpu):
            beta = np.float32(jnp.maximum(jnp.mean(jnp.abs(jnp.asarray(W))), EPS))
    except Exception:
        beta = np.float32(max(np.abs(W).astype(np.float64).mean(), EPS))

    wq = np.clip(np.round(W / beta), np.float32(-1), np.float32(1)).astype(
        np.float32
    )
    return beta, wq


def kernel(x: np.ndarray, W: np.ndarray, b: np.ndarray) -> np.ndarray:
    out, _ = _run(x, W, b)
    return out


def _run(x, W, b, **spmd_kwargs):
    x = np.ascontiguousarray(np.asarray(x, dtype=np.float32))
    W = np.ascontiguousarray(np.asarray(W, dtype=np.float32))
    b = np.ascontiguousarray(np.asarray(b, dtype=np.float32))

    B, T, KI = x.shape
    OC_full, KI2 = W.shape
    assert KI == KI2 == IN_FEATURES and OC_full == OUT_FEATURES
    assert KB + K8 == IN_FEATURES
    NT = B * T
    assert NT == N_TOKENS

    TC = NT // S_WAYS  # tokens per core
    OC = OUT_FEATURES // Q_WAYS  # out features per core

    beta, wq = _host_quant(W)
    S = np.float32(beta * LAYER_SCALE)

    wqt = wq.T  # [KI, OUT]
    xf = x.reshape(NT, KI).T  # [KI, NT]
    xb_full = np.ascontiguousarray(xf[:KB]).astype(ml_dtypes.bfloat16)
    x8_full = np.ascontiguousarray(xf[KB:]).astype(ml_dtypes.float8_e4m3)

    xb_s = [
        np.ascontiguousarray(xb_full[:, s * TC : (s + 1) * TC])
        for s in range(S_WAYS)
    ]
    x8_s = [
        np.ascontiguousarray(x8_full[:, s * TC : (s + 1) * TC])
        for s in range(S_WAYS)
    ]
    wb_q = [
        np.ascontiguousarray(wqt[:KB, q * OC : (q + 1) * OC]).astype(
            ml_dtypes.bfloat16
        )
        for q in range(Q_WAYS)
    ]
    w8_q = [
        np.ascontiguousarray(wqt[KB:, q * OC : (q + 1) * OC]).astype(
            ml_dtypes.float8_e4m3
        )
        for q in range(Q_WAYS)
    ]
    # device expects cb[p, 0] = beta*LS and cb[p, 1 + m] = b_shard[m*128 + p]
    m_tiles = OC // P
    cb_q = []
    for q in range(Q_WAYS):
        cba = np.empty((P, 1 + m_tiles), dtype=np.float32)
        cba[:, 0] = S
        cba[:, 1:] = b[q * OC : (q + 1) * OC].reshape(m_tiles, P).T
        cb_q.append(np.ascontiguousarray(cba))

    in_maps = []
    for s in range(S_WAYS):
        for q in range(Q_WAYS):
            in_maps.append(
                {
                    "xb": xb_s[s],
                    "x8": x8_s[s],
                    "wb": wb_q[q],
                    "w8": w8_q[q],
                    "cb": cb_q[q],
                }
            )

    nc = build_nc(OC, TC)
    res = run_bass_kernel_spmd(
        nc, in_maps, core_ids=list(range(N_CORES)), **spmd_kwargs
    )

    out_full = np.empty((NT, OUT_FEATURES), dtype=np.float32)
    for s in range(S_WAYS):
        for q in range(Q_WAYS):
            piece = res.results[s * Q_WAYS + q]["out"]  # [OC, TC]
            out_full[s * TC : (s + 1) * TC, q * OC : (q + 1) * OC] = piece.T
    return out_full.reshape(B, T, OUT_FEATURES), res
